# revision 4
# baseline (speedup 1.0000x reference)
"""Trainium2 Bass kernel for nn_HCRLayer (Legendre tensor-product density layer).

out[b,o] = 1 + sum_{idx != 0} C[o,idx] * prod_j sqrt(2*i_j+1) * P_{i_j}(2*x_bj - 1)

Math notes:
  - basis[:, :, 0] == 1.0 exactly (P_0 = 1, scale sqrt(1) = 1), so the "+1" and
    the zero-term fold exactly into the coefficient matrix: with
    pouter[b,k] = prod_j P_{i_j}(t_bj)  (raw Legendre, no scales; pouter[b,0] = 1),
    C3[k,o] = scale_k * c_flat[o,k] for k != 0 and C3[0,o] = 1.0,
    we get out = pouter @ C3 exactly.
  - pouter factorizes: k = 36*p + q, pouter = t01[b,p] * t23[b,q] where
    t01 = P_{i0}(t0)*P_{i1}(t1) (36 vals), t23 = P_{i2}(t2)*P_{i3}(t3).

Device scheme (per core, data-parallel over batch, 8 cores):
  - sample-major build of basis/t01/t23 on VectorE (broadcast APs),
  - PE transpose of [t01|t23] into feature-major TFM [72, 512] per superblock,
  - per K-tile (12 tiles of 108 = 3*36 features, 1296 = 12*108 exactly):
      selector matmul replicates t01 rows across partitions (PSUM),
      VectorE multiply with the tile-invariant t23 pattern T23P (SBUF),
      main matmul vs C3 tile accumulates the output in PSUM.
  - outputs stored o-major [32, 16384] per core; host transposes (free).
"""

import sys
import numpy as np

sys.path.insert(0, "/opt/trn_rl_repo")

# ---- problem constants (hardcoded per contract) ----
B, D, O = 131072, 4, 32
M1 = 6                      # m+1 basis functions per dim
NCORES = 8
BC = B // NCORES            # 16384 samples per core
SBW = 512                   # samples per superblock (matmul moving width)
NSB = BC // SBW             # 32 superblocks
NBLK = SBW // 128           # 4 partition-blocks per superblock
NT = 12                     # K tiles
KT = 108                    # features per K tile (3*36); 12*108 = 1296
NK = 1296

_CACHE = {}


def _host_consts(coefficients: np.ndarray):
    """C3 [1296, 32] (scales + bias folded), selectors, identity."""
    s = np.sqrt(2.0 * np.arange(M1) + 1.0)
    scale_k = np.einsum("i,j,k,l->ijkl", s, s, s, s).reshape(NK).astype(np.float64)
    c_flat = coefficients.reshape(O, NK).astype(np.float64)
    C3 = (c_flat * scale_k[None, :]).T.astype(np.float32).copy()  # [1296, 32]
    C3[0, :] = 1.0
    c3t = np.ascontiguousarray(
        C3.reshape(NT, KT, O).transpose(1, 0, 2).reshape(KT, NT * O)
    )  # [108, NT*32]; per-tile lhsT = c3t[:, t*32:(t+1)*32]

    sel1 = np.zeros((36, NT * KT), np.float32)  # t01 replication selectors
    for t in range(NT):
        for kp in range(KT):
            sel1[3 * t + kp // 36, t * KT + kp] = 1.0
    selq = np.zeros((36, KT), np.float32)       # t23 pattern selector (tile-invariant)
    for kp in range(KT):
        selq[kp % 36, kp] = 1.0
    ident = np.eye(128, dtype=np.float32)
    return c3t, sel1, selq, ident


def _build_program():
    from contextlib import ExitStack
    from concourse import bacc, mybir, tile

    F32 = mybir.dt.float32
    ALU = mybir.AluOpType

    nc = bacc.Bacc("TRN2", target_bir_lowering=False, enable_partition_id=False)

    xprep_d = nc.dram_tensor("xprep", [128, NSB * NBLK * D], F32, kind="ExternalInput")
    ident_d = nc.dram_tensor("ident", [128, 128], F32, kind="ExternalInput")
    sel1_d = nc.dram_tensor("sel1", [36, NT * KT], F32, kind="ExternalInput")
    selq_d = nc.dram_tensor("selq", [36, KT], F32, kind="ExternalInput")
    c3t_d = nc.dram_tensor("c3t", [KT, NT * O], F32, kind="ExternalInput")
    out_d = nc.dram_tensor("out", [O, BC], F32, kind="ExternalOutput")

    NG = NSB * NBLK          # 128 partition-blocks per core
    with tile.TileContext(nc) as tc:
        with ExitStack() as ctx:
            cpool = ctx.enter_context(tc.tile_pool(name="const", bufs=1))
            apool = ctx.enter_context(tc.tile_pool(name="pha", bufs=1))
            wpool = ctx.enter_context(tc.tile_pool(name="work", bufs=2))
            opool = ctx.enter_context(tc.tile_pool(name="outer", bufs=3))
            ps_t_pool = ctx.enter_context(tc.tile_pool(name="ps_t", bufs=2, space="PSUM"))
            ps_q_pool = ctx.enter_context(tc.tile_pool(name="ps_q", bufs=2, space="PSUM"))
            ps_x_pool = ctx.enter_context(tc.tile_pool(name="ps_x", bufs=2, space="PSUM"))
            ps_o_pool = ctx.enter_context(tc.tile_pool(name="ps_o", bufs=2, space="PSUM"))

            xprep = cpool.tile([128, NG * D], F32)
            ident = cpool.tile([128, 128], F32)
            sel1 = cpool.tile([36, NT * KT], F32)
            selq_full = cpool.tile([100, KT], F32)
            selq = selq_full[64:100, :]
            c3t = cpool.tile([KT, NT * O], F32)
            nc.sync.dma_start(xprep[:], xprep_d[:])
            nc.sync.dma_start(ident[:], ident_d[:])
            nc.sync.dma_start(sel1[:], sel1_d[:])
            nc.sync.dma_start(selq, selq_d[:])
            nc.sync.dma_start(c3t[:], c3t_d[:])

            # ---------------- Phase A: basis / t01 / t23 (sample-major) ----------
            NGJ = NG * D      # 512 (group, j) pairs
            tt = apool.tile([128, NGJ], F32)       # t = 2x - 1
            tmp = apool.tile([128, NGJ], F32)      # t^2
            u = apool.tile([128, NGJ], F32)
            v = apool.tile([128, NGJ], F32)
            basis = apool.tile([128, NGJ * M1], F32)   # [(g,j), i]
            GW = 100  # group width: t01 at 0:36, t23 at 64:100 (base-partition rule)
            tsm = apool.tile([128, NG * GW], F32)

            nc.vector.tensor_scalar(tt[:], xprep[:], 2.0, 1.0, ALU.mult, ALU.subtract)
            nc.vector.tensor_tensor(tmp[:], tt[:], tt[:], ALU.mult)

            bview = basis[:].rearrange("p (g i) -> p g i", i=M1)
            # P0 = 1  (t*0 + 1)
            nc.vector.tensor_scalar(bview[:, :, 0], tt[:], 0.0, 1.0, ALU.mult, ALU.add)
            # P1 = t
            nc.vector.tensor_copy(bview[:, :, 1], tt[:])
            # P2 = 1.5 t^2 - 0.5
            nc.vector.tensor_scalar(bview[:, :, 2], tmp[:], 1.5, 0.5, ALU.mult, ALU.subtract)
            # P3 = t (2.5 t^2 - 1.5)
            nc.vector.tensor_scalar(u[:], tmp[:], 2.5, 1.5, ALU.mult, ALU.subtract)
            nc.vector.tensor_tensor(bview[:, :, 3], u[:], tt[:], ALU.mult)
            # P4 = (4.375 t^2 - 3.75) t^2 + 0.375
            nc.vector.tensor_scalar(u[:], tmp[:], 4.375, 3.75, ALU.mult, ALU.subtract)
            nc.vector.tensor_tensor(v[:], u[:], tmp[:], ALU.mult)
            nc.vector.tensor_scalar(bview[:, :, 4], v[:], 1.0, 0.375, ALU.mult, ALU.add)
            # P5 = t ((7.875 t^2 - 8.75) t^2 + 1.875)
            nc.vector.tensor_scalar(u[:], tmp[:], 7.875, 8.75, ALU.mult, ALU.subtract)
            nc.vector.tensor_tensor(v[:], u[:], tmp[:], ALU.mult)
            nc.vector.tensor_scalar(u[:], v[:], 1.0, 1.875, ALU.mult, ALU.add)
            nc.vector.tensor_tensor(bview[:, :, 5], u[:], tt[:], ALU.mult)

            # t01/t23 outer products via broadcast APs into tsm
            b4 = basis[:].rearrange("p (g j i) -> p g j i", j=D, i=M1)  # g = NG
            tview = tsm[:].rearrange("p (g k) -> p g k", k=GW)
            nc.vector.memset(tview[:, :, 36:64], 0.0)  # padding read by transposes
            for (ja, jb, k0) in ((0, 1, 0), (2, 3, 64)):
                a0 = b4[:, :, ja, :].unsqueeze(3).broadcast_to([128, NG, M1, M1])
                a1 = b4[:, :, jb, :].unsqueeze(2).broadcast_to([128, NG, M1, M1])
                out36 = tview[:, :, k0 : k0 + 36].rearrange(
                    "p g (a b) -> p g a b", a=M1, b=M1
                )
                nc.vector.tensor_tensor(out36, a0, a1, ALU.mult)

            # ---------------- Phase B: per superblock --------------------------
            outsb = apool.tile([O, BC], F32)
            for sb in range(NSB):
                ps_T = ps_t_pool.tile([100, SBW], F32)
                for blk in range(NBLK):
                    g = sb * NBLK + blk
                    nc.tensor.transpose(
                        ps_T[:, blk * 128 : (blk + 1) * 128],
                        tsm[:, g * GW : (g + 1) * GW],
                        ident[:],
                    )
                tfm = wpool.tile([100, SBW], F32, tag="tfm")
                nc.scalar.copy(tfm[:], ps_T[:])

                ps_Q = ps_q_pool.tile([KT, SBW], F32)
                nc.tensor.matmul(ps_Q[:], selq, tfm[64:100, :], start=True, stop=True)
                t23p = wpool.tile([KT, SBW], F32, tag="t23p")
                nc.scalar.copy(t23p[:], ps_Q[:])

                ps_O = ps_o_pool.tile([O, SBW], F32)
                for t in range(NT):
                    ps_X = ps_x_pool.tile([KT, SBW], F32)
                    nc.tensor.matmul(
                        ps_X[:], sel1[:, t * KT : (t + 1) * KT], tfm[0:36, :],
                        start=True, stop=True,
                    )
                    outer = opool.tile([KT, SBW], F32, tag="outer")
                    nc.vector.tensor_tensor(outer[:], ps_X[:], t23p[:], ALU.mult)
                    nc.tensor.matmul(
                        ps_O[:], c3t[:, t * O : (t + 1) * O], outer[:],
                        start=(t == 0), stop=(t == NT - 1),
                    )
                nc.scalar.copy(outsb[:, sb * SBW : (sb + 1) * SBW], ps_O[:])

            nc.sync.dma_start(out_d[:], outsb[:])

    nc.compile()
    return nc


def kernel(x: np.ndarray, coefficients: np.ndarray) -> np.ndarray:
    from concourse.bass_utils import run_bass_kernel_spmd

    x = np.ascontiguousarray(np.asarray(x, dtype=np.float32))
    coefficients = np.ascontiguousarray(np.asarray(coefficients, dtype=np.float32))
    assert x.shape == (B, D) and coefficients.shape == (O,) + (M1,) * D

    if "nc" not in _CACHE:
        _CACHE["nc"] = _build_program()
    nc = _CACHE["nc"]

    c3t, sel1, selq, ident = _host_consts(coefficients)

    # x layout: sample s (within core) = sb*512 + blk*128 + p
    xs = x.reshape(NCORES, NSB, NBLK, 128, D)
    xprep = np.ascontiguousarray(
        xs.transpose(0, 3, 1, 2, 4).reshape(NCORES, 128, NSB * NBLK * D)
    )

    in_maps = [
        {
            "xprep": xprep[c],
            "ident": ident,
            "sel1": sel1,
            "selq": selq,
            "c3t": c3t,
        }
        for c in range(NCORES)
    ]
    res = run_bass_kernel_spmd(
        nc, in_maps, core_ids=list(range(NCORES)), trace=bool(_CACHE.get("trace"))
    )
    _CACHE["last_res"] = res
    # per-core out: [32, 16384] (o, sample) -> full [B, 32]
    out = np.empty((B, O), np.float32)
    for c in range(NCORES):
        out[c * BC : (c + 1) * BC, :] = res.results[c]["out"].T
    return out


if __name__ == "__main__":
    rng = np.random.default_rng(0)
    x = rng.uniform(size=(B, D)).astype(np.float32)
    coeff = rng.standard_normal((O,) + (M1,) * D).astype(np.float32)
    y = kernel(x, coeff)
    print("kernel output", y.shape, y.dtype, float(np.abs(y).mean()))


# revision 5
# speedup vs baseline: 1.4396x; 1.4396x over previous
"""Trainium2 Bass kernel for nn_HCRLayer (Legendre tensor-product density layer).

out[b,o] = 1 + sum_{idx != 0} C[o,idx] * prod_j sqrt(2*i_j+1) * P_{i_j}(2*x_bj - 1)

Math notes:
  - basis[:, :, 0] == 1.0 exactly (P_0 = 1, scale sqrt(1) = 1), so the "+1" and
    the zero-term fold exactly into the coefficient matrix: with
    pouter[b,k] = prod_j P_{i_j}(t_bj)  (raw Legendre, no scales; pouter[b,0] = 1),
    C3[k,o] = scale_k * c_flat[o,k] for k != 0 and C3[0,o] = 1.0,
    we get out = pouter @ C3 exactly.
  - pouter factorizes: k = 36*p + q, pouter = t01[b,p] * t23[b,q] where
    t01 = P_{i0}(t0)*P_{i1}(t1) (36 vals), t23 = P_{i2}(t2)*P_{i3}(t3).

Device scheme (per core, data-parallel over batch, 8 cores):
  - sample-major build of basis/t01/t23 on VectorE (broadcast APs),
  - PE transpose of [t01|t23] into feature-major TFM [72, 512] per superblock,
  - per K-tile (12 tiles of 108 = 3*36 features, 1296 = 12*108 exactly):
      selector matmul replicates t01 rows across partitions (PSUM),
      VectorE multiply with the tile-invariant t23 pattern T23P (SBUF),
      main matmul vs C3 tile accumulates the output in PSUM.
  - outputs stored o-major [32, 16384] per core; host transposes (free).
"""

import sys
import numpy as np

sys.path.insert(0, "/opt/trn_rl_repo")

# ---- problem constants (hardcoded per contract) ----
B, D, O = 131072, 4, 32
M1 = 6                      # m+1 basis functions per dim
NCORES = 8
BC = B // NCORES            # 16384 samples per core
SBW = 512                   # samples per superblock (matmul moving width)
NSB = BC // SBW             # 32 superblocks
NBLK = SBW // 128           # 4 partition-blocks per superblock
NT = 12                     # K tiles
KT = 108                    # features per K tile (3*36); 12*108 = 1296
NK = 1296

_CACHE = {}


def _round_fp32r(v: np.ndarray) -> np.ndarray:
    """Round fp32 to the PE's fp32r input format (11 mantissa bits)."""
    u = v.astype(np.float32).view(np.uint32).astype(np.uint64)
    u = ((u + 0x800) & 0xFFFFF000) & 0xFFFFFFFF
    return u.astype(np.uint32).view(np.float32)


def _host_consts(coefficients: np.ndarray):
    """C3 [1296, 32] (scales + bias folded), selectors, identity."""
    s = np.sqrt(2.0 * np.arange(M1) + 1.0)
    scale_k = np.einsum("i,j,k,l->ijkl", s, s, s, s).reshape(NK).astype(np.float64)
    c_flat = coefficients.reshape(O, NK).astype(np.float64)
    C3 = (c_flat * scale_k[None, :]).T.astype(np.float32).copy()  # [1296, 32]
    C3[0, :] = 1.0
    c3t = np.ascontiguousarray(
        C3.reshape(NT, KT, O).transpose(1, 0, 2).reshape(KT, NT * O)
    )  # [108, NT*32]; per-tile lhsT = c3t[:, t*32:(t+1)*32]
    c3t = _round_fp32r(c3t)  # device consumes it as float32r

    sel1 = np.zeros((36, NT * KT), np.float32)  # t01 replication selectors
    for t in range(NT):
        for kp in range(KT):
            sel1[3 * t + kp // 36, t * KT + kp] = 1.0
    selq = np.zeros((36, KT), np.float32)       # t23 pattern selector (tile-invariant)
    for kp in range(KT):
        selq[kp % 36, kp] = 1.0
    ident = np.eye(128, dtype=np.float32)
    return c3t, sel1, selq, ident


def _build_program():
    from contextlib import ExitStack
    from concourse import bacc, mybir, tile

    F32 = mybir.dt.float32
    F32R = mybir.dt.float32r
    ALU = mybir.AluOpType

    nc = bacc.Bacc("TRN2", target_bir_lowering=False, enable_partition_id=False)

    xprep_d = nc.dram_tensor("xprep", [128, NSB * NBLK * D], F32, kind="ExternalInput")
    ident_d = nc.dram_tensor("ident", [128, 128], F32, kind="ExternalInput")
    sel1_d = nc.dram_tensor("sel1", [36, NT * KT], F32R, kind="ExternalInput")
    selq_d = nc.dram_tensor("selq", [36, KT], F32R, kind="ExternalInput")
    c3t_d = nc.dram_tensor("c3t", [KT, NT * O], F32R, kind="ExternalInput")
    out_d = nc.dram_tensor("out", [O, BC], F32, kind="ExternalOutput")

    NG = NSB * NBLK          # 128 partition-blocks per core
    with tile.TileContext(nc) as tc:
        with ExitStack() as ctx:
            cpool = ctx.enter_context(tc.tile_pool(name="const", bufs=1))
            apool = ctx.enter_context(tc.tile_pool(name="pha", bufs=1))
            wpool = ctx.enter_context(tc.tile_pool(name="work", bufs=2))
            opool = ctx.enter_context(tc.tile_pool(name="outer", bufs=3))
            ps_t_pool = ctx.enter_context(tc.tile_pool(name="ps_t", bufs=2, space="PSUM"))
            ps_q_pool = ctx.enter_context(tc.tile_pool(name="ps_q", bufs=2, space="PSUM"))
            ps_x_pool = ctx.enter_context(tc.tile_pool(name="ps_x", bufs=2, space="PSUM"))
            ps_o_pool = ctx.enter_context(tc.tile_pool(name="ps_o", bufs=2, space="PSUM"))

            xprep = cpool.tile([128, NG * D], F32)
            ident = cpool.tile([128, 128], F32)
            sel1 = cpool.tile([36, NT * KT], F32R)
            selq_full = cpool.tile([100, KT], F32R)
            selq = selq_full[64:100, :]
            c3t = cpool.tile([KT, NT * O], F32R)
            nc.sync.dma_start(xprep[:], xprep_d[:])
            nc.sync.dma_start(ident[:], ident_d[:])
            nc.sync.dma_start(sel1[:], sel1_d[:])
            nc.sync.dma_start(selq, selq_d[:])
            nc.sync.dma_start(c3t[:], c3t_d[:])

            # ---------------- Phase A: basis / t01 / t23 (sample-major) ----------
            NGJ = NG * D      # 512 (group, j) pairs
            tt = apool.tile([128, NGJ], F32)       # t = 2x - 1
            tmp = apool.tile([128, NGJ], F32)      # t^2
            u = apool.tile([128, NGJ], F32)
            v = apool.tile([128, NGJ], F32)
            basis = apool.tile([128, NGJ * M1], F32)   # [(g,j), i]
            GW = 100  # group width: t01 at 0:36, t23 at 64:100 (base-partition rule)
            tsm = apool.tile([128, NG * GW], F32)

            nc.vector.tensor_scalar(tt[:], xprep[:], 2.0, 1.0, ALU.mult, ALU.subtract)
            nc.vector.tensor_tensor(tmp[:], tt[:], tt[:], ALU.mult)

            bview = basis[:].rearrange("p (g i) -> p g i", i=M1)
            # P0 = 1  (t*0 + 1)
            nc.vector.tensor_scalar(bview[:, :, 0], tt[:], 0.0, 1.0, ALU.mult, ALU.add)
            # P1 = t
            nc.vector.tensor_copy(bview[:, :, 1], tt[:])
            # P2 = 1.5 t^2 - 0.5
            nc.vector.tensor_scalar(bview[:, :, 2], tmp[:], 1.5, 0.5, ALU.mult, ALU.subtract)
            # P3 = t (2.5 t^2 - 1.5)
            nc.vector.tensor_scalar(u[:], tmp[:], 2.5, 1.5, ALU.mult, ALU.subtract)
            nc.vector.tensor_tensor(bview[:, :, 3], u[:], tt[:], ALU.mult)
            # P4 = (4.375 t^2 - 3.75) t^2 + 0.375
            nc.vector.tensor_scalar(u[:], tmp[:], 4.375, 3.75, ALU.mult, ALU.subtract)
            nc.vector.tensor_tensor(v[:], u[:], tmp[:], ALU.mult)
            nc.vector.tensor_scalar(bview[:, :, 4], v[:], 1.0, 0.375, ALU.mult, ALU.add)
            # P5 = t ((7.875 t^2 - 8.75) t^2 + 1.875)
            nc.vector.tensor_scalar(u[:], tmp[:], 7.875, 8.75, ALU.mult, ALU.subtract)
            nc.vector.tensor_tensor(v[:], u[:], tmp[:], ALU.mult)
            nc.vector.tensor_scalar(u[:], v[:], 1.0, 1.875, ALU.mult, ALU.add)
            nc.vector.tensor_tensor(bview[:, :, 5], u[:], tt[:], ALU.mult)

            # t01/t23 outer products via broadcast APs into tsm
            b4 = basis[:].rearrange("p (g j i) -> p g j i", j=D, i=M1)  # g = NG
            tview = tsm[:].rearrange("p (g k) -> p g k", k=GW)
            nc.vector.memset(tview[:, :, 36:64], 0.0)  # padding read by transposes
            for (ja, jb, k0) in ((0, 1, 0), (2, 3, 64)):
                a0 = b4[:, :, ja, :].unsqueeze(3).broadcast_to([128, NG, M1, M1])
                a1 = b4[:, :, jb, :].unsqueeze(2).broadcast_to([128, NG, M1, M1])
                out36 = tview[:, :, k0 : k0 + 36].rearrange(
                    "p g (a b) -> p g a b", a=M1, b=M1
                )
                nc.vector.tensor_tensor(out36, a0, a1, ALU.mult)

            # ---------------- Phase B: per superblock --------------------------
            outsb = apool.tile([O, BC], F32)
            for sb in range(NSB):
                ps_T = ps_t_pool.tile([100, SBW], F32)
                for blk in range(NBLK):
                    g = sb * NBLK + blk
                    nc.tensor.transpose(
                        ps_T[:, blk * 128 : (blk + 1) * 128],
                        tsm[:, g * GW : (g + 1) * GW],
                        ident[:],
                    )
                tfm = wpool.tile([100, SBW], F32R, tag="tfm")
                nc.scalar.copy(tfm[:], ps_T[:])

                ps_Q = ps_q_pool.tile([KT, SBW], F32)
                nc.tensor.matmul(ps_Q[:], selq, tfm[64:100, :], start=True, stop=True)
                t23p = wpool.tile([KT, SBW], F32R, tag="t23p")
                nc.scalar.copy(t23p[:], ps_Q[:])

                ps_O = ps_o_pool.tile([O, SBW], F32)
                for t in range(NT):
                    ps_X = ps_x_pool.tile([KT, SBW], F32)
                    nc.tensor.matmul(
                        ps_X[:], sel1[:, t * KT : (t + 1) * KT], tfm[0:36, :],
                        start=True, stop=True,
                    )
                    outer = opool.tile([KT, SBW], F32R, tag="outer")
                    nc.vector.tensor_tensor(outer[:], ps_X[:], t23p[:], ALU.mult)
                    nc.tensor.matmul(
                        ps_O[:], c3t[:, t * O : (t + 1) * O], outer[:],
                        start=(t == 0), stop=(t == NT - 1),
                    )
                nc.scalar.copy(outsb[:, sb * SBW : (sb + 1) * SBW], ps_O[:])

            nc.sync.dma_start(out_d[:], outsb[:])

    nc.compile()
    return nc


def kernel(x: np.ndarray, coefficients: np.ndarray) -> np.ndarray:
    from concourse.bass_utils import run_bass_kernel_spmd

    x = np.ascontiguousarray(np.asarray(x, dtype=np.float32))
    coefficients = np.ascontiguousarray(np.asarray(coefficients, dtype=np.float32))
    assert x.shape == (B, D) and coefficients.shape == (O,) + (M1,) * D

    if "nc" not in _CACHE:
        _CACHE["nc"] = _build_program()
    nc = _CACHE["nc"]

    c3t, sel1, selq, ident = _host_consts(coefficients)

    # x layout: sample s (within core) = sb*512 + blk*128 + p
    xs = x.reshape(NCORES, NSB, NBLK, 128, D)
    xprep = np.ascontiguousarray(
        xs.transpose(0, 3, 1, 2, 4).reshape(NCORES, 128, NSB * NBLK * D)
    )

    in_maps = [
        {
            "xprep": xprep[c],
            "ident": ident,
            "sel1": sel1,
            "selq": selq,
            "c3t": c3t,
        }
        for c in range(NCORES)
    ]
    res = run_bass_kernel_spmd(
        nc, in_maps, core_ids=list(range(NCORES)), trace=bool(_CACHE.get("trace"))
    )
    _CACHE["last_res"] = res
    # per-core out: [32, 16384] (o, sample) -> full [B, 32]
    out = np.empty((B, O), np.float32)
    for c in range(NCORES):
        out[c * BC : (c + 1) * BC, :] = res.results[c]["out"].T
    return out


if __name__ == "__main__":
    rng = np.random.default_rng(0)
    x = rng.uniform(size=(B, D)).astype(np.float32)
    coeff = rng.standard_normal((O,) + (M1,) * D).astype(np.float32)
    y = kernel(x, coeff)
    print("kernel output", y.shape, y.dtype, float(np.abs(y).mean()))
